# revision 1
# baseline (speedup 1.0000x reference)
"""LIF spike kernel for Trainium2 (Bass/Tile), data-parallel over batch on 8 cores.

Host layout per core: x_core [C=128, B_loc=4, T*HW=8192] f32 (contiguous),
so DMA-in per-partition runs are 16KB. Output uint8 [128, 4, 8192], converted
to f32 on host (spikes are exactly 0/1).

Per timestep t (on [128, 1024] column slices):
  u_t     = (neg_mem * -TAU) + x_t     DVE scalar_tensor_tensor (t=0: u_0 = x_0)
  spike_t = u_t > THRESH  (uint8)      gpsimd tensor_scalar is_gt
  neg_mem = (spike_t - 1) * u_t        DVE scalar_tensor_tensor (u8/f32 mixed)
"""

import numpy as np

import concourse.bacc as bacc
import concourse.mybir as mybir
from concourse.tile import TileContext
from concourse.bass_utils import run_bass_kernel_spmd

B, T, C, H, W = 32, 8, 128, 32, 32
HW = H * W
N_CORES = 8
B_LOC = B // N_CORES
TAU = 0.5
THRESH = 1.0

# engine per op, tunable: 'v' = vector (DVE), 'g' = gpsimd
SPIKE_ENG = ["v"] * T
NEGMEM_ENG = ["v"] * (T - 1)
U_ENG = ["v"] * (T - 1)  # index t-1 for t in 1..7

_nc_cache = None


def build_nc():
    nc = bacc.Bacc("TRN2", target_bir_lowering=False)
    f32 = mybir.dt.float32
    u8 = mybir.dt.uint8
    op = mybir.AluOpType
    x = nc.dram_tensor("x", [C, B_LOC, T * HW], f32, kind="ExternalInput")
    out = nc.dram_tensor("out", [C, B_LOC, T * HW], u8, kind="ExternalOutput")

    def eng(code):
        return nc.vector if code == "v" else nc.gpsimd

    with TileContext(nc) as tc:
        with (
            tc.tile_pool(name="xp", bufs=8) as xp,
            tc.tile_pool(name="op_", bufs=3) as opool,
            tc.tile_pool(name="up", bufs=4) as up,
            tc.tile_pool(name="mp", bufs=4) as mp,
        ):
            for b in range(B_LOC):
                xc = []
                for j in range(4):
                    xt = xp.tile([C, 2 * HW], f32, tag="xc")
                    nc.sync.dma_start(
                        out=xt[:], in_=x[:, b, j * 2 * HW : (j + 1) * 2 * HW]
                    )
                    xc.append(xt)
                ob = opool.tile([C, T * HW], u8, tag="ob")
                negmem = None
                for t in range(T):
                    xs = xc[t // 2][:, (t % 2) * HW : (t % 2 + 1) * HW]
                    if t == 0:
                        u = xs
                    else:
                        ut = up.tile([C, HW], f32, tag="u")
                        eng(U_ENG[t - 1]).scalar_tensor_tensor(
                            ut[:], negmem[:], -TAU, xs, op.mult, op.add
                        )
                        u = ut[:]
                    sp = ob[:, t * HW : (t + 1) * HW]
                    eng(SPIKE_ENG[t]).tensor_scalar(sp, u, THRESH, None, op.is_gt)
                    if t < T - 1:
                        negmem = mp.tile([C, HW], f32, tag="nm")
                        eng(NEGMEM_ENG[t]).scalar_tensor_tensor(
                            negmem[:], sp, 1.0, u, op.subtract, op.mult
                        )
                nc.sync.dma_start(out=out[:, b], in_=ob[:])
    nc.compile()
    return nc


def make_in_maps(x: np.ndarray) -> list[dict]:
    xs = np.ascontiguousarray(x).reshape(B, T, C, HW)
    return [
        {
            "x": np.ascontiguousarray(
                xs[i * B_LOC : (i + 1) * B_LOC].transpose(2, 0, 1, 3)
            ).reshape(C, B_LOC, T * HW)
        }
        for i in range(N_CORES)
    ]


def kernel(x: np.ndarray) -> np.ndarray:
    global _nc_cache
    if _nc_cache is None:
        _nc_cache = build_nc()
    res = run_bass_kernel_spmd(_nc_cache, make_in_maps(x), list(range(N_CORES)))
    # out[c, b_loc, t*HW+hw] -> [b, t, c, hw]
    parts = [
        res.results[i]["out"].reshape(C, B_LOC, T, HW).transpose(1, 2, 0, 3)
        for i in range(N_CORES)
    ]
    full = np.concatenate(parts, axis=0)
    return full.reshape(B, T, C, H, W).astype(np.float32)

